# revision 3
# baseline (speedup 1.0000x reference)
"""CrossAttention Trainium2 Bass kernel.

Problem: B=2, Q=S=2048, D=1024, H=16 heads, A=64 head_dim.
  q = (iQ @ Wq)   -> [B,H,Q,A]
  k,v = iK @ Wkv  -> [B,H,S,A] each
  scores = q k^T / 8, mask -> -1e9, softmax over S
  out = (attn @ v) @ Wo -> [B,Q,D]

Sharding: 8 cores = 2 batches x 4 head-groups (4 heads each).
Each core computes a partial [Q, D] = ctx_local @ Wo_rows(local heads);
host sums the 4 partials per batch (row-parallel Wo unshard).

Device layout trick: everything is computed "transposed" (feature dim on
partitions) so no on-device transposes are needed:
  - host ships iQ^T, iK^T (pre-tiled [128, 8, 2048])
  - qT[a,q], kT[a,s] from matmul(lhsT=W, rhs=iX^T)
  - scoresT[s,q] = matmul(lhsT=kT_slice, rhs=qT)       (K=64 contraction)
  - exp via scalar activation, mask bias is a per-partition bias AP
  - V kept natural [s,a] with an appended ones column -> attn@V matmul
    also yields the softmax denominator row for free
  - ctxT normalized via reciprocal + K=1 outer-product broadcast
  - out[q,n] = matmul(lhsT=ctxT_h tile, rhs=Wo_h rows), accum over heads
"""

import sys
import numpy as np

for _p in ("/opt/trn_rl_repo",):
    if _p not in sys.path:
        sys.path.insert(0, _p)

import ml_dtypes

B, Q, S, D = 2, 2048, 2048, 1024
H, A = 16, 64
HG = 4            # heads per core
NCORES = 8
NEG = -1e9

_cache = {}


def _build_program():
    import concourse.bass as bass  # noqa
    import concourse.bacc as bacc
    import concourse.tile as tile
    from concourse import mybir

    f32 = mybir.dt.float32
    bf16 = mybir.dt.bfloat16
    EXP = mybir.ActivationFunctionType.Exp
    MULT = mybir.AluOpType.mult

    nc = bacc.Bacc("TRN2", target_bir_lowering=False, debug=False)

    iqt = nc.dram_tensor("iqt", [128, 8, Q], bf16, kind="ExternalInput").ap()
    ikt = nc.dram_tensor("ikt", [128, 8, S], bf16, kind="ExternalInput").ap()
    wq = nc.dram_tensor("wq", [128, 8, 256], bf16, kind="ExternalInput").ap()
    wk = nc.dram_tensor("wk", [128, 8, 256], bf16, kind="ExternalInput").ap()
    wv = nc.dram_tensor("wv", [128, 8, 256], bf16, kind="ExternalInput").ap()
    wo = nc.dram_tensor("wo", [64, HG, D], bf16, kind="ExternalInput").ap()
    mb = nc.dram_tensor("mb", [128, 16], f32, kind="ExternalInput").ap()
    out = nc.dram_tensor("out", [128, 16, D], f32, kind="ExternalOutput").ap()

    NQT = Q // 128          # 16 q tiles
    NST = S // 128          # 16 s tiles
    NDT = D // 128          # 8 d tiles

    with tile.TileContext(nc) as tc:
        with (
            tc.tile_pool(name="persist", bufs=1) as persist,
            tc.tile_pool(name="expp", bufs=3) as expp,
            tc.tile_pool(name="outp", bufs=3) as outp,
            tc.tile_pool(name="srp", bufs=2) as srp,
            tc.tile_pool(name="scp", bufs=2, space="PSUM") as scp,
            tc.tile_pool(name="ctxp", bufs=1, space="PSUM") as ctxp,
            tc.tile_pool(name="bcp", bufs=1, space="PSUM") as bcp,
        ):
            # ---- persistent loads ----
            iqt_sb = persist.tile([128, 8, Q], bf16, tag="iqt")
            nc.sync.dma_start(iqt_sb[:], iqt[:])
            ikt_sb = persist.tile([128, 8, S], bf16, tag="ikt")
            nc.sync.dma_start(ikt_sb[:], ikt[:])
            wq_sb = persist.tile([128, 8, 256], bf16, tag="wq")
            nc.sync.dma_start(wq_sb[:], wq[:])
            wk_sb = persist.tile([128, 8, 256], bf16, tag="wk")
            nc.sync.dma_start(wk_sb[:], wk[:])
            wv_sb = persist.tile([128, 8, 256], bf16, tag="wv")
            nc.sync.dma_start(wv_sb[:], wv[:])
            wo_sb = persist.tile([64, HG, D], bf16, tag="wo")
            nc.sync.dma_start(wo_sb[:], wo[:])
            mb_sb = persist.tile([128, 16], f32, tag="mb")
            nc.sync.dma_start(mb_sb[:], mb[:])

            ones_sb = persist.tile([1, 64], f32, tag="ones")
            nc.vector.memset(ones_sb[:], 1.0)

            qt_sb = persist.tile([128, 2, Q], bf16, tag="qt")
            kt_sb = persist.tile([128, 2, S], bf16, tag="kt")
            v_sb = persist.tile([128, NST, HG, 65], bf16, tag="v")
            nc.vector.memset(v_sb[:, :, :, 64:65], 1.0)
            ctxn = [
                persist.tile([64, Q], bf16, tag=f"ctxn{h}", name=f"ctxn{h}")
                for h in range(HG)
            ]

            # ---- projections: qT [a, q] and kT [a, s] ----
            for wsb, xsb, osb in ((wq_sb, iqt_sb, qt_sb), (wk_sb, ikt_sb, kt_sb)):
                for at in range(2):          # 128-wide slab of the 256 head cols
                    for qc in range(2):      # 1024-wide chunk of q/s
                        ps = scp.tile([128, 1024], mybir.dt.float32, tag="mm")
                        for c in range(2):   # 512-col matmul chunks
                            for dt_i in range(NDT):
                                nc.tensor.matmul(
                                    ps[:, c * 512:(c + 1) * 512],
                                    lhsT=wsb[:, dt_i, at * 128:(at + 1) * 128],
                                    rhs=xsb[:, dt_i,
                                            qc * 1024 + c * 512:qc * 1024 + (c + 1) * 512],
                                    start=(dt_i == 0),
                                    stop=(dt_i == NDT - 1),
                                )
                        nc.scalar.copy(
                            out=osb[:, at, qc * 1024:(qc + 1) * 1024], in_=ps[:]
                        )

            # ---- V projection: natural [s, a] per head (+ ones col kept) ----
            for st in range(NST):
                ps = scp.tile([128, HG, 64], mybir.dt.float32, tag="mm")
                for dt_i in range(NDT):
                    nc.tensor.matmul(
                        ps[:],
                        lhsT=ikt_sb[:, dt_i, st * 128:(st + 1) * 128],
                        rhs=wv_sb[:, dt_i, :],
                        start=(dt_i == 0),
                        stop=(dt_i == NDT - 1),
                    )
                nc.scalar.copy(out=v_sb[:, st, :, 0:64], in_=ps[:])

            # ---- attention per (head, q-chunk) ----
            for h in range(HG):
                po = (h % 2) * 64
                ti = h // 2
                for qc in range(2):
                    q0 = qc * 1024
                    ctx = ctxp.tile([65, 1024], mybir.dt.float32, tag="ctx")
                    for st in range(NST):
                        sc = scp.tile([128, 1024], mybir.dt.float32, tag="mm")
                        for c in range(2):
                            nc.tensor.matmul(
                                sc[:, c * 512:(c + 1) * 512],
                                lhsT=kt_sb[po:po + 64, ti, st * 128:(st + 1) * 128],
                                rhs=qt_sb[po:po + 64, ti,
                                          q0 + c * 512:q0 + (c + 1) * 512],
                                start=True,
                                stop=True,
                            )
                        ex = expp.tile([128, 1024], bf16, tag="exp")
                        nc.scalar.activation(
                            out=ex[:], in_=sc[:], func=EXP,
                            bias=mb_sb[:, st:st + 1], scale=0.125,
                        )
                        for c in range(2):
                            nc.tensor.matmul(
                                ctx[:, c * 512:(c + 1) * 512],
                                lhsT=v_sb[:, st, h, :],
                                rhs=ex[:, c * 512:(c + 1) * 512],
                                start=(st == 0),
                                stop=(st == NST - 1),
                            )
                    # normalize: ctx[:64] / ctx[64]
                    recip = srp.tile([1, 1024], mybir.dt.float32, tag="recip")
                    nc.vector.reciprocal(recip[:], ctx[64:65, :])
                    bc = bcp.tile([64, 1024], mybir.dt.float32, tag="bc")
                    for c in range(2):
                        nc.tensor.matmul(
                            bc[:, c * 512:(c + 1) * 512],
                            lhsT=ones_sb[:],
                            rhs=recip[:, c * 512:(c + 1) * 512],
                            start=True,
                            stop=True,
                        )
                    bcs = srp.tile([64, 1024], mybir.dt.float32, tag="bcs")
                    nc.scalar.copy(out=bcs[:], in_=bc[:])
                    nc.vector.tensor_tensor(
                        ctxn[h][:, q0:q0 + 1024], ctx[0:64, :], bcs[:], MULT
                    )

            # ---- output projection: out[q, n] partial over local heads ----
            for qt in range(NQT):
                ps = scp.tile([128, 1024], mybir.dt.float32, tag="mm")
                for c in range(2):
                    for h in range(HG):
                        nc.tensor.matmul(
                            ps[:, c * 512:(c + 1) * 512],
                            lhsT=ctxn[h][:, qt * 128:(qt + 1) * 128],
                            rhs=wo_sb[:, h, c * 512:(c + 1) * 512],
                            start=(h == 0),
                            stop=(h == HG - 1),
                        )
                ob = outp.tile([128, 1024], mybir.dt.float32, tag="ob")
                nc.scalar.copy(out=ob[:], in_=ps[:])
                nc.sync.dma_start(out[:, qt, :], ob[:])

    nc.compile()
    return nc


def _get_program():
    if "nc" not in _cache:
        _cache["nc"] = _build_program()
    return _cache["nc"]


def _prep_inputs(iQ, iK, mask, Wq, Wkv, Wo):
    """Build the 8 per-core input maps (host-side shard + layout + cast)."""
    bf = ml_dtypes.bfloat16
    iQ = np.asarray(iQ, dtype=np.float32)
    iK = np.asarray(iK, dtype=np.float32)
    mask = np.asarray(mask)
    Wq = np.asarray(Wq, dtype=np.float32)
    Wkv = np.asarray(Wkv, dtype=np.float32)
    Wo = np.asarray(Wo, dtype=np.float32)

    def tile_kxn(a):  # [K=1024, N] -> [128, K/128, N]
        K, N = a.shape
        return np.ascontiguousarray(
            a.reshape(K // 128, 128, N).transpose(1, 0, 2)
        )

    in_maps = []
    per_b = {}
    for b in range(B):
        per_b[b] = {
            "iqt": tile_kxn(iQ[b].T).astype(bf),
            "ikt": tile_kxn(iK[b].T).astype(bf),
            "mb": np.ascontiguousarray(
                np.where(mask[b, 0], np.float32(NEG), np.float32(0.0))
                .astype(np.float32).reshape(16, 128).T
            ),
        }
    for c in range(NCORES):
        b, g = divmod(c, NCORES // B)
        cols = slice(g * 256, (g + 1) * 256)
        wo_g = Wo[g * 256:(g + 1) * 256, :]          # [256, 1024]
        in_maps.append({
            "iqt": per_b[b]["iqt"],
            "ikt": per_b[b]["ikt"],
            "mb": per_b[b]["mb"],
            "wq": tile_kxn(Wq[:, cols]).astype(bf),
            "wk": tile_kxn(Wkv[:, cols]).astype(bf),
            "wv": tile_kxn(Wkv[:, 1024 + g * 256:1024 + (g + 1) * 256]).astype(bf),
            "wo": np.ascontiguousarray(
                wo_g.reshape(HG, 64, D).transpose(1, 0, 2)
            ).astype(bf),
        })
    return in_maps


def _run(inputs, trace=False):
    from concourse.bass_utils import run_bass_kernel_spmd

    nc = _get_program()
    in_maps = _prep_inputs(**inputs)
    res = run_bass_kernel_spmd(
        nc, in_maps, list(range(NCORES)), trace=trace
    )
    outs = []
    for b in range(B):
        acc = None
        for g in range(NCORES // B):
            o = np.asarray(
                res.results[b * (NCORES // B) + g]["out"], dtype=np.float32
            )
            acc = o if acc is None else acc + o
        # [128, 16, 1024] -> [2048, 1024]
        outs.append(acc.transpose(1, 0, 2).reshape(Q, D))
    return np.stack(outs), res


def kernel(**inputs):
    out, _ = _run(inputs, trace=False)
    return out
